# revision 10
# baseline (speedup 1.0000x reference)
"""Trainium2 Bass kernel for CompressedLinear:
    out = x @ (weight_int8 * scale[:, None]).T + bias

Strategy:
  - Data-parallel over tokens: x [4,2048,4096] -> [8192,4096] -> 8 shards
    of [1024,4096], one per NeuronCore. Weight/scale/bias replicated.
  - Per core: out_c[o, t] = sum_k w[o,k] * x_c[t,k], then *scale[o] + bias[o].
  - Pure bf16 matmul: weights are int8-valued (exact in bf16); x is cast
    to bf16 on the host (rel err ~1e-3, budget 2e-2). bf16 halves x HBM
    traffic vs fp32 and enables the compiler's fast-weight-load (FWL)
    path, which f32r (4-byte reads) blocks -- LDWEIGHTS at ~half cost so
    the per-matmul issue gap stays at the N=512 streaming floor.
  - Weight stationary [128k x 128o] tiles, x moving [128k x 512t] blocks.
  - Output-feature tiles processed in groups of 3 with the k-loop
    interleaved across the group (6 PSUM banks rotating through all 8),
    so the PE has ~41us of work per group and overlaps the initial x load.
  - Weights stream bf16 straight into the working pool (no staging, no
    cast), half-blocks (kt 0-15 / 16-31) one group ahead on the sync
    queue; x rides the scalar HW-DGE queue.
  - Warm-up matmuls on memset tiles at t=0 hold the PE busy so the HAM
    clock-gate opens (1.2->2.4 GHz) before the first real data lands.
  - Last group runs ot-serial / tb-major so its PSUM evictions and
    output stores overlap the remaining matmuls instead of serializing
    after the final one.
  - Fused scale+bias on PSUM eviction (DVE tensor_scalar / ACT Identity
    alternating), output [o, t] per core, host-side gather/transpose.
"""

import numpy as np

B, S, IN, OUT = 4, 2048, 4096, 4096
N_CORES = 8
TOK = (B * S) // N_CORES  # 1024 tokens per core
P = 128
KT = IN // P   # 32 k-tiles
OT = OUT // P  # 32 output-feature tiles
NB = 512       # moving free dim per matmul
TB = TOK // NB  # 2 token blocks
# x SBUF chunk sizes in k-tiles: small first chunks so the first matmul
# fires as soon as the first ~0.25MB of x has landed.
XCHUNKS = [1, 1, 2, 4, 4, 4, 4, 4, 4, 4]
WARM_MMS = 6   # dummy matmuls to hold the PE HAM clock-gate open at startup
KH = KT // 2   # half-block kt extent (16)
GRP = 3        # output-feature tiles per interleaved group

_PROG = None  # (nc, names)


def _build():
    import concourse.mybir as mybir
    import concourse.tile as tile
    from concourse import bacc

    f32 = mybir.dt.float32
    bf16 = mybir.dt.bfloat16

    groups = [list(range(g, min(g + GRP, OT))) for g in range(0, OT, GRP)]
    assert sum(XCHUNKS) == KT
    # kt -> (chunk index, offset inside chunk)
    kt_map = {}
    _kt = 0
    for ci, sz in enumerate(XCHUNKS):
        for off in range(sz):
            kt_map[_kt] = (ci, off)
            _kt += 1

    nc = bacc.Bacc(None, target_bir_lowering=False, debug=False)
    with tile.TileContext(nc) as tc:
        with tc.tile_pool(name="dram", bufs=1, space="DRAM") as dram:
            xT_d = dram.tile([P, KT, TOK], bf16, kind="ExternalInput", name="xT")
            w_d = dram.tile([OT, P, KT, P], bf16, kind="ExternalInput", name="w")
            sc_d = dram.tile([P, OT], f32, kind="ExternalInput", name="sc")
            bi_d = dram.tile([P, OT], f32, kind="ExternalInput", name="bi")
            out_d = dram.tile([P, OT, TOK], f32, kind="ExternalOutput", name="out")

            with (
                tc.tile_pool(name="const", bufs=1) as constp,
                tc.tile_pool(name="xp", bufs=1) as xp,
                tc.tile_pool(name="wp", bufs=4 * GRP) as wp,
                tc.tile_pool(name="wq", bufs=4 * GRP) as wqp,
                tc.tile_pool(name="op", bufs=4) as outp,
                tc.tile_pool(name="ps", bufs=8, space="PSUM") as psp,
            ):
                sc_sb = constp.tile([P, OT], f32, tag="sc")
                bi_sb = constp.tile([P, OT], f32, tag="bi")

                # wk[(ot, kt)] -> (sbuf tile, kt offset within tile)
                wk = {}

                def w_dma(ot, h, eng=None):
                    t = wp.tile([P, KH, P], bf16, tag="w", name=f"w{ot}h{h}")
                    (eng or nc.sync).dma_start(
                        t[:], w_d[ot, :, h * KH : (h + 1) * KH, :]
                    )
                    for j in range(KH):
                        wk[(ot, h * KH + j)] = (t, j)

                def w_dma_q(ot, q):
                    # group-0 startup: 4-kt quarter tiles so each ot's first
                    # weights land in ~0.4us instead of ~1.5us
                    t = wqp.tile([P, 4, P], bf16, tag="wq", name=f"wq{ot}q{q}")
                    nc.sync.dma_start(t[:], w_d[ot, :, q * 4 : (q + 1) * 4, :])
                    for j in range(4):
                        wk[(ot, q * 4 + j)] = (t, j)

                x_tiles = []

                def x_dma(i, eng=None):
                    sz = XCHUNKS[i]
                    k0 = sum(XCHUNKS[:i])
                    t = xp.tile([P, sz, TOK], bf16, tag=f"x{i}", name=f"x{i}")
                    (eng or nc.scalar).dma_start(t[:], xT_d[:, k0 : k0 + sz, :])
                    x_tiles.append(t)

                # Startup order: x chunks stream on the scalar queue from t=0;
                # weights stream on the sync queue concurrently.
                # scale/bias aren't needed until the first eviction.
                x_dma(0)
                x_dma(1)
                for q in range(KH // 4):
                    for ot in groups[0]:
                        w_dma_q(ot, q)
                for i in range(2, len(XCHUNKS)):
                    x_dma(i)
                for ot in groups[0]:
                    w_dma(ot, 1)
                nc.scalar.dma_start(sc_sb[:], sc_d[:])
                nc.scalar.dma_start(bi_sb[:], bi_d[:])

                if WARM_MMS:
                    # Warm-up: dummy bf16 matmuls on memset tiles keep the PE
                    # busy so the HAM clock-gate opens (1.2->2.4 GHz) before
                    # the first real matmul's data lands.
                    wu_w = constp.tile([P, P], bf16, tag="wu_w")
                    wu_x = constp.tile([P, NB], bf16, tag="wu_x")
                    nc.vector.memset(wu_w[:], 0.0)
                    nc.vector.memset(wu_x[:], 0.0)
                    wu_ps = [
                        psp.tile([P, NB], f32, tag="ps", name=f"wu_ps{i}")
                        for i in range(2)
                    ]
                    for i in range(WARM_MMS):
                        nc.tensor.matmul(
                            wu_ps[i % 2][:], wu_w[:], wu_x[:], start=True, stop=True
                        )

                evict_n = [0]

                def evict(ps_t, ot, tb):
                    o_sb = outp.tile([P, NB], f32, tag="o", name="o_sb")
                    if evict_n[0] % 2 == 0:
                        nc.vector.tensor_scalar(
                            o_sb[:],
                            ps_t[:],
                            sc_sb[:, ot : ot + 1],
                            bi_sb[:, ot : ot + 1],
                            op0=mybir.AluOpType.mult,
                            op1=mybir.AluOpType.add,
                        )
                    else:
                        # out = Identity(in*scale + bias) on ScalarE;
                        # splits eviction across two engines.
                        nc.scalar.activation(
                            o_sb[:],
                            ps_t[:],
                            mybir.ActivationFunctionType.Identity,
                            bias=bi_sb[:, ot : ot + 1],
                            scale=sc_sb[:, ot : ot + 1],
                        )
                    evict_n[0] += 1
                    # scalar HW-DGE queue (free after x loads): avoids
                    # the costly SWDGE drain that gpsimd DMAs incur
                    nc.scalar.dma_start(
                        out_d[:, ot, tb * NB : (tb + 1) * NB], o_sb[:]
                    )

                for gi, group in enumerate(groups):
                    last_group = gi == len(groups) - 1
                    # Prefetch next group's weights (h0 then h1). Group 1's
                    # prefetch rides the scalar queue BEHIND all x chunks:
                    # queue serialization keeps it from stealing HBM
                    # bandwidth from x while group 0's k-loop consumes it.
                    if gi + 1 < len(groups):
                        peng = nc.scalar if gi == 0 else None
                        for h in range(2):
                            for ot in groups[gi + 1]:
                                w_dma(ot, h, eng=peng)
                    ps = {}
                    for ot in group:
                        for tb in range(TB):
                            ps[(ot, tb)] = psp.tile(
                                [P, NB], f32, tag="ps", name=f"ps{ot}_{tb}"
                            )

                    def mm1(ot, kt, tb):
                        wt, khi = wk[(ot, kt)]
                        ci, off = kt_map[kt]
                        nc.tensor.matmul(
                            ps[(ot, tb)][:],
                            wt[:, khi, :],
                            x_tiles[ci][:, off, tb * NB : (tb + 1) * NB],
                            start=(kt == 0),
                            stop=(kt == KT - 1),
                        )

                    def mm(ot, kt):
                        for tb in range(TB):
                            mm1(ot, kt, tb)

                    if last_group:
                        # ot-serial / tb-major: each (ot, tb) finishes its
                        # k-loop early and evicts while later matmuls run,
                        # so only the very last eviction+store trails the
                        # final matmul. That last one is split in half
                        # across DVE+ACT and stored on two queues to halve
                        # the trailing latency.
                        for oi, ot in enumerate(group):
                            for tb in range(TB):
                                for kt in range(KT):
                                    mm1(ot, kt, tb)
                                if oi + 1 < len(group) or tb + 1 < TB:
                                    evict(ps[(ot, tb)], ot, tb)
                                    continue
                                ps_t = ps[(ot, tb)]
                                HB = NB // 2
                                oA = outp.tile([P, HB], f32, tag="o", name="oA")
                                oB = outp.tile([P, HB], f32, tag="o", name="oB")
                                nc.vector.tensor_scalar(
                                    oA[:],
                                    ps_t[:, :HB],
                                    sc_sb[:, ot : ot + 1],
                                    bi_sb[:, ot : ot + 1],
                                    op0=mybir.AluOpType.mult,
                                    op1=mybir.AluOpType.add,
                                )
                                nc.scalar.activation(
                                    oB[:],
                                    ps_t[:, HB:],
                                    mybir.ActivationFunctionType.Identity,
                                    bias=bi_sb[:, ot : ot + 1],
                                    scale=sc_sb[:, ot : ot + 1],
                                )
                                t0 = tb * NB
                                nc.sync.dma_start(
                                    out_d[:, ot, t0 : t0 + HB], oA[:]
                                )
                                nc.scalar.dma_start(
                                    out_d[:, ot, t0 + HB : t0 + NB], oB[:]
                                )
                        continue

                    if gi == 0:
                        # Staggered entry: ot0 runs kt 0-3 alone (only needs
                        # w00h0 + the first x chunks), then ot1/ot2 catch up
                        # while their weight blocks arrive.
                        for ot in group:
                            for kt in range(4):
                                mm(ot, kt)
                        kt_start = 4
                    else:
                        kt_start = 0
                    for kt in range(kt_start, KT):
                        for ot in group:
                            mm(ot, kt)
                    for ot in group:
                        for tb in range(TB):
                            evict(ps[(ot, tb)], ot, tb)
    nc.compile()
    names = {
        "xT": xT_d.tensor.name,
        "w": w_d.tensor.name,
        "sc": sc_d.tensor.name,
        "bi": bi_d.tensor.name,
        "out": out_d.tensor.name,
    }
    return nc, names


def _get_prog():
    global _PROG
    if _PROG is None:
        _PROG = _build()
    return _PROG


def _marshal(x, weight_int8, scale, bias):
    import ml_dtypes

    # weight [o, k] -> [ot, p(k), kt, ol]; bf16 is exact for int8 values
    w = np.asarray(weight_int8, dtype=np.float32).astype(ml_dtypes.bfloat16)
    w_m = np.ascontiguousarray(
        w.reshape(OT, P, KT, P).transpose(0, 3, 2, 1)
    )
    sc_m = np.ascontiguousarray(np.asarray(scale, np.float32).reshape(OT, P).T)
    bi_m = np.ascontiguousarray(np.asarray(bias, np.float32).reshape(OT, P).T)
    x_flat = np.asarray(x, np.float32).astype(ml_dtypes.bfloat16).reshape(B * S, IN)
    x_shards = []
    for c in range(N_CORES):
        sh = x_flat[c * TOK : (c + 1) * TOK]  # [t, k]
        x_shards.append(
            np.ascontiguousarray(sh.reshape(TOK, KT, P).transpose(2, 1, 0))
        )
    return w_m, sc_m, bi_m, x_shards


def _run(x, weight_int8, scale, bias, trace=False):
    from concourse.bass_utils import run_bass_kernel_spmd

    nc, names = _get_prog()
    w_m, sc_m, bi_m, x_shards = _marshal(x, weight_int8, scale, bias)
    in_maps = [
        {
            names["xT"]: x_shards[c],
            names["w"]: w_m,
            names["sc"]: sc_m,
            names["bi"]: bi_m,
        }
        for c in range(N_CORES)
    ]
    res = run_bass_kernel_spmd(
        nc, in_maps, core_ids=list(range(N_CORES)), trace=trace
    )
    full = np.empty((B * S, OUT), dtype=np.float32)
    for c in range(N_CORES):
        out_c = res.results[c][names["out"]]  # [p, ot, t]
        full[c * TOK : (c + 1) * TOK] = out_c.transpose(2, 1, 0).reshape(TOK, OUT)
    return full.reshape(B, S, OUT), res


def kernel(x, weight_int8, scale, bias):
    out, _ = _run(x, weight_int8, scale, bias, trace=False)
    return out


def kernel_traced(x, weight_int8, scale, bias):
    out, res = _run(x, weight_int8, scale, bias, trace=True)
    return out, res


# revision 18
# speedup vs baseline: 1.0921x; 1.0921x over previous
"""Trainium2 Bass kernel for CompressedLinear:
    out = x @ (weight_int8 * scale[:, None]).T + bias

Strategy:
  - Data-parallel over tokens: x [4,2048,4096] -> [8192,4096] -> 8 shards
    of [1024,4096], one per NeuronCore. Weight/scale/bias replicated.
  - Per core: out_c[o, t] = sum_k w[o,k] * x_c[t,k], then *scale[o] + bias[o].
  - Pure bf16 matmul: weights are int8-valued (exact in bf16); x is cast
    to bf16 on the host (rel err ~1e-3, budget 2e-2). bf16 halves x HBM
    traffic vs fp32 and enables the compiler's fast-weight-load (FWL)
    path, which f32r (4-byte reads) blocks -- LDWEIGHTS at ~half cost so
    the per-matmul issue gap stays at the N=512 streaming floor.
  - Weight stationary [128k x 128o] tiles, x moving [128k x 512t] blocks.
  - Output-feature tiles processed in groups of 3 with the k-loop
    interleaved across the group (6 PSUM banks rotating through all 8),
    so the PE has ~41us of work per group and overlaps the initial x load.
  - Weights stream bf16 straight into the working pool (no staging, no
    cast), half-blocks (kt 0-15 / 16-31) one group ahead on the sync
    queue; x rides the scalar HW-DGE queue.
  - Warm-up matmuls on memset tiles at t=0 hold the PE busy so the HAM
    clock-gate opens (1.2->2.4 GHz) before the first real data lands.
  - Last group runs ot-serial / tb-major so its PSUM evictions and
    output stores overlap the remaining matmuls instead of serializing
    after the final one.
  - Fused scale+bias on PSUM eviction (DVE tensor_scalar / ACT Identity
    alternating), output [o, t] per core, host-side gather/transpose.
"""

import numpy as np

B, S, IN, OUT = 4, 2048, 4096, 4096
N_CORES = 8
TOK = (B * S) // N_CORES  # 1024 tokens per core
P = 128
KT = IN // P   # 32 k-tiles
OT = OUT // P  # 32 output-feature tiles
NB = 512       # moving free dim per matmul
TB = TOK // NB  # 2 token blocks
# x SBUF chunk sizes in k-tiles: small first chunks so the first matmul
# fires as soon as the first ~0.25MB of x has landed.
XCHUNKS = [1, 1, 2, 4, 4, 4, 4, 4, 4, 4]
WARM_MMS = 6   # dummy matmuls to hold the PE HAM clock-gate open at startup
KH = KT // 2   # half-block kt extent (16)
GRP = 3        # output-feature tiles per interleaved group

_PROG = None  # (nc, names)


def _build():
    import concourse.mybir as mybir
    import concourse.tile as tile
    from concourse import bacc

    f32 = mybir.dt.float32
    bf16 = mybir.dt.bfloat16

    # Group 0 gets 4 ots (8 PSUM banks, ~55us of PE work) so the full x
    # stream can land during it without starving the PE; then groups of 3;
    # a single-ot final group keeps the eviction tail minimal.
    groups = [[0, 1, 2, 3]]
    groups += [list(range(g, g + 3)) for g in range(4, OT - 1, 3)]
    groups += [[OT - 1]]
    assert sorted(sum(groups, [])) == list(range(OT))
    assert sum(XCHUNKS) == KT
    # kt -> (chunk index, offset inside chunk)
    kt_map = {}
    _kt = 0
    for ci, sz in enumerate(XCHUNKS):
        for off in range(sz):
            kt_map[_kt] = (ci, off)
            _kt += 1

    nc = bacc.Bacc(None, target_bir_lowering=False, debug=False)
    with tile.TileContext(nc) as tc:
        with tc.tile_pool(name="dram", bufs=1, space="DRAM") as dram:
            xT_d = dram.tile([P, KT, TOK], bf16, kind="ExternalInput", name="xT")
            w_d = dram.tile([OT, P, KT, P], bf16, kind="ExternalInput", name="w")
            sc_d = dram.tile([P, OT], f32, kind="ExternalInput", name="sc")
            bi_d = dram.tile([P, OT], f32, kind="ExternalInput", name="bi")
            out_d = dram.tile([P, OT, TOK], f32, kind="ExternalOutput", name="out")

            with (
                tc.tile_pool(name="const", bufs=1) as constp,
                tc.tile_pool(name="xp", bufs=1) as xp,
                tc.tile_pool(name="wp", bufs=4 * GRP) as wp,
                tc.tile_pool(name="wq", bufs=16) as wqp,
                tc.tile_pool(name="op", bufs=4) as outp,
                tc.tile_pool(name="ps", bufs=8, space="PSUM") as psp,
            ):
                sc_sb = constp.tile([P, OT], f32, tag="sc")
                bi_sb = constp.tile([P, OT], f32, tag="bi")

                # wk[(ot, kt)] -> (sbuf tile, kt offset within tile)
                wk = {}

                def w_dma(ot, h, eng=None):
                    t = wp.tile([P, KH, P], bf16, tag="w", name=f"w{ot}h{h}")
                    (eng or nc.sync).dma_start(
                        t[:], w_d[ot, :, h * KH : (h + 1) * KH, :]
                    )
                    for j in range(KH):
                        wk[(ot, h * KH + j)] = (t, j)

                def w_dma_q(ot, q):
                    # group-0 startup: 4-kt quarter tiles so each ot's first
                    # weights land in ~0.4us instead of ~1.5us
                    t = wqp.tile([P, 4, P], bf16, tag="wq", name=f"wq{ot}q{q}")
                    nc.sync.dma_start(t[:], w_d[ot, :, q * 4 : (q + 1) * 4, :])
                    for j in range(4):
                        wk[(ot, q * 4 + j)] = (t, j)

                x_tiles = []

                def x_dma(i):
                    # alternate the two HW-DGE queues (scalar/sync) so x
                    # keeps its share of HBM bandwidth against the weight
                    # stream, in consumption order on both queues
                    sz = XCHUNKS[i]
                    k0 = sum(XCHUNKS[:i])
                    t = xp.tile([P, sz, TOK], bf16, tag=f"x{i}", name=f"x{i}")
                    eng = nc.scalar if i % 2 == 0 else nc.sync
                    eng.dma_start(t[:], xT_d[:, k0 : k0 + sz, :])
                    x_tiles.append(t)

                # Startup order: x chunks stream on the scalar queue from t=0;
                # weights stream on the sync queue concurrently.
                # scale/bias aren't needed until the first eviction.
                # Deadline-ordered interleave: x chunks and group-0 weight
                # quarters alternate on both queues in the order the PE
                # will consume them.
                x_dma(0)
                for ot in groups[0]:
                    w_dma_q(ot, 0)
                x_dma(1)
                for ot in groups[0]:
                    w_dma_q(ot, 1)
                x_dma(2)
                x_dma(3)
                for ot in groups[0]:
                    w_dma_q(ot, 2)
                x_dma(4)
                x_dma(5)
                for ot in groups[0]:
                    w_dma_q(ot, 3)
                x_dma(6)
                x_dma(7)
                nc.scalar.dma_start(sc_sb[:], sc_d[:])
                nc.scalar.dma_start(bi_sb[:], bi_d[:])
                for ot in groups[0]:
                    w_dma(ot, 1)
                x_dma(8)
                x_dma(9)

                if WARM_MMS:
                    # Warm-up: dummy bf16 matmuls on memset tiles keep the PE
                    # busy so the HAM clock-gate opens (1.2->2.4 GHz) before
                    # the first real matmul's data lands.
                    wu_w = constp.tile([P, P], bf16, tag="wu_w")
                    wu_x = constp.tile([P, NB], bf16, tag="wu_x")
                    nc.vector.memset(wu_w[:], 0.0)
                    nc.vector.memset(wu_x[:], 0.0)
                    wu_ps = [
                        psp.tile([P, NB], f32, tag="ps", name=f"wu_ps{i}")
                        for i in range(2)
                    ]
                    for i in range(WARM_MMS):
                        nc.tensor.matmul(
                            wu_ps[i % 2][:], wu_w[:], wu_x[:], start=True, stop=True
                        )

                evict_n = [0]

                def evict(ps_t, ot, tb):
                    o_sb = outp.tile([P, NB], f32, tag="o", name="o_sb")
                    if evict_n[0] % 2 == 0:
                        nc.vector.tensor_scalar(
                            o_sb[:],
                            ps_t[:],
                            sc_sb[:, ot : ot + 1],
                            bi_sb[:, ot : ot + 1],
                            op0=mybir.AluOpType.mult,
                            op1=mybir.AluOpType.add,
                        )
                    else:
                        # out = Identity(in*scale + bias) on ScalarE;
                        # splits eviction across two engines.
                        nc.scalar.activation(
                            o_sb[:],
                            ps_t[:],
                            mybir.ActivationFunctionType.Identity,
                            bias=bi_sb[:, ot : ot + 1],
                            scale=sc_sb[:, ot : ot + 1],
                        )
                    evict_n[0] += 1
                    # scalar HW-DGE queue (free after x loads): avoids
                    # the costly SWDGE drain that gpsimd DMAs incur
                    nc.scalar.dma_start(
                        out_d[:, ot, tb * NB : (tb + 1) * NB], o_sb[:]
                    )

                for gi, group in enumerate(groups):
                    last_group = gi == len(groups) - 1
                    # Prefetch next group's weights (h0 then h1). Group 1's
                    # prefetch rides the scalar queue BEHIND all x chunks:
                    # queue serialization keeps it from stealing HBM
                    # bandwidth from x while group 0's k-loop consumes it.
                    if gi + 1 < len(groups):
                        # group 1's prefetch rides BEHIND the x chunks on
                        # the scalar/vector queues so it can't steal HBM
                        # bandwidth from x while group 0 consumes it
                        for h in range(2):
                            for oi, ot in enumerate(groups[gi + 1]):
                                peng = (
                                    (nc.scalar if oi % 2 == 0 else nc.sync)
                                    if gi == 0
                                    else None
                                )
                                w_dma(ot, h, eng=peng)
                    ps = {}
                    for ot in group:
                        for tb in range(TB):
                            ps[(ot, tb)] = psp.tile(
                                [P, NB], f32, tag="ps", name=f"ps{ot}_{tb}"
                            )

                    def mm1(ot, kt, tb):
                        wt, khi = wk[(ot, kt)]
                        ci, off = kt_map[kt]
                        nc.tensor.matmul(
                            ps[(ot, tb)][:],
                            wt[:, khi, :],
                            x_tiles[ci][:, off, tb * NB : (tb + 1) * NB],
                            start=(kt == 0),
                            stop=(kt == KT - 1),
                        )

                    def mm(ot, kt):
                        for tb in range(TB):
                            mm1(ot, kt, tb)

                    if last_group:
                        # ot-serial / tb-major: each (ot, tb) finishes its
                        # k-loop early and evicts while later matmuls run,
                        # so only the very last eviction+store trails the
                        # final matmul. That last one is split in half
                        # across DVE+ACT and stored on two queues to halve
                        # the trailing latency.
                        for oi, ot in enumerate(group):
                            for tb in range(TB):
                                for kt in range(KT):
                                    mm1(ot, kt, tb)
                                if oi + 1 < len(group) or tb + 1 < TB:
                                    evict(ps[(ot, tb)], ot, tb)
                                    continue
                                ps_t = ps[(ot, tb)]
                                HB = NB // 2
                                oA = outp.tile([P, HB], f32, tag="o", name="oA")
                                oB = outp.tile([P, HB], f32, tag="o", name="oB")
                                nc.vector.tensor_scalar(
                                    oA[:],
                                    ps_t[:, :HB],
                                    sc_sb[:, ot : ot + 1],
                                    bi_sb[:, ot : ot + 1],
                                    op0=mybir.AluOpType.mult,
                                    op1=mybir.AluOpType.add,
                                )
                                nc.scalar.activation(
                                    oB[:],
                                    ps_t[:, HB:],
                                    mybir.ActivationFunctionType.Identity,
                                    bias=bi_sb[:, ot : ot + 1],
                                    scale=sc_sb[:, ot : ot + 1],
                                )
                                t0 = tb * NB
                                nc.sync.dma_start(
                                    out_d[:, ot, t0 : t0 + HB], oA[:]
                                )
                                nc.scalar.dma_start(
                                    out_d[:, ot, t0 + HB : t0 + NB], oB[:]
                                )
                        continue

                    if gi == 0:
                        # Staggered entry: ot0 runs kt 0-3 alone (only needs
                        # w00h0 + the first x chunks), then ot1/ot2 catch up
                        # while their weight blocks arrive.
                        for ot in group:
                            for kt in range(4):
                                mm(ot, kt)
                        kt_start = 4
                    else:
                        kt_start = 0
                    for kt in range(kt_start, KT):
                        for ot in group:
                            mm(ot, kt)
                    for ot in group:
                        for tb in range(TB):
                            evict(ps[(ot, tb)], ot, tb)
    nc.compile()
    names = {
        "xT": xT_d.tensor.name,
        "w": w_d.tensor.name,
        "sc": sc_d.tensor.name,
        "bi": bi_d.tensor.name,
        "out": out_d.tensor.name,
    }
    return nc, names


def _get_prog():
    global _PROG
    if _PROG is None:
        _PROG = _build()
    return _PROG


def _marshal(x, weight_int8, scale, bias):
    import ml_dtypes

    # weight [o, k] -> [ot, p(k), kt, ol]; bf16 is exact for int8 values
    w = np.asarray(weight_int8, dtype=np.float32).astype(ml_dtypes.bfloat16)
    w_m = np.ascontiguousarray(
        w.reshape(OT, P, KT, P).transpose(0, 3, 2, 1)
    )
    sc_m = np.ascontiguousarray(np.asarray(scale, np.float32).reshape(OT, P).T)
    bi_m = np.ascontiguousarray(np.asarray(bias, np.float32).reshape(OT, P).T)
    x_flat = np.asarray(x, np.float32).astype(ml_dtypes.bfloat16).reshape(B * S, IN)
    x_shards = []
    for c in range(N_CORES):
        sh = x_flat[c * TOK : (c + 1) * TOK]  # [t, k]
        x_shards.append(
            np.ascontiguousarray(sh.reshape(TOK, KT, P).transpose(2, 1, 0))
        )
    return w_m, sc_m, bi_m, x_shards


def _run(x, weight_int8, scale, bias, trace=False):
    from concourse.bass_utils import run_bass_kernel_spmd

    nc, names = _get_prog()
    w_m, sc_m, bi_m, x_shards = _marshal(x, weight_int8, scale, bias)
    in_maps = [
        {
            names["xT"]: x_shards[c],
            names["w"]: w_m,
            names["sc"]: sc_m,
            names["bi"]: bi_m,
        }
        for c in range(N_CORES)
    ]
    res = run_bass_kernel_spmd(
        nc, in_maps, core_ids=list(range(N_CORES)), trace=trace
    )
    full = np.empty((B * S, OUT), dtype=np.float32)
    for c in range(N_CORES):
        out_c = res.results[c][names["out"]]  # [p, ot, t]
        full[c * TOK : (c + 1) * TOK] = out_c.transpose(2, 1, 0).reshape(TOK, OUT)
    return full.reshape(B, S, OUT), res


def kernel(x, weight_int8, scale, bias):
    out, _ = _run(x, weight_int8, scale, bias, trace=False)
    return out


def kernel_traced(x, weight_int8, scale, bias):
    out, res = _run(x, weight_int8, scale, bias, trace=True)
    return out, res
